# revision 85
# baseline (speedup 1.0000x reference)
"""Trainium2 Bass kernel for nn_AutoregressivePredictor (8-step greedy rollout
of a single Llama decoder layer over 32 independent time steps).

Strategy (TP8 across the 8 NeuronCores):
  - core c owns q-heads [4c..4c+4), kv-head c, FF slice [1792c..1792c+1792),
    and the W_out column slice for features Fc = {128r + p : p in [16c,16c+16)}
    (the natural ReduceScatter split of the [128, KT*T] mlp partial buffer).
  - weights are streamed from HBM in fp16 (halves DMA vs fp32); moving
    activations are fp16 so PE runs 1 cycle/row.  The logits matmul is full
    fp32 (resident column-sharded W_out) to keep argmax margins.
  - collectives per step: ReduceScatter+AllGather (fp32) for the Wo partial
    sum, ReduceScatter (fp32, with xp/8 folded into the payload) for the Wd
    partial, and one AllReduce of the [V=1024, T] logits followed by a local
    argmax on every core (no cross-core argmax dance).
  - DMA queues: weight streams issue on SP only, software-pipelined with
    explicit prefetch (next step's leading tiles issue right after this
    step's last consumption, so they stream under the logits/argmax/emb
    tail); collective-adjacent stores/loads issue on Activation; the
    xT accumulate uses the gpsimd software-DGE accum path.
"""
import numpy as np

import concourse.bass as bass
import concourse.mybir as mybir
import concourse.tile as tile
from concourse import bacc
from concourse.bass_utils import run_bass_kernel_spmd
from concourse.masks import make_identity

P = 128
D, NH, NKV, HD, FF, V, T, GEN = 4096, 32, 8, 128, 14336, 1024, 32, 8
NCORES = 8
ROPE_THETA = 500000.0
EPS = 1e-5
KT = D // P            # 32 k-tiles over the model dim
QH = NH // NCORES      # 4 q heads per core
FFC = FF // NCORES     # 1792 ff features per core
FKT = FFC // P         # 14 k-tiles over the ff shard
VC = V // P            # 8 vocab chunks of 128
F32 = mybir.dt.float32
F16 = mybir.dt.float16

_CACHED = {}


def _build_nc():
    nc = bacc.Bacc("TRN2", target_bir_lowering=False, debug=False,
                   num_devices=NCORES)

    # ---- inputs (per-core, pre-packed on host) ----
    wqkv = nc.dram_tensor("wqkv", [6, P, KT * P], F16, kind="ExternalInput")
    wo = nc.dram_tensor("wo", [KT, P, 4 * P], F16, kind="ExternalInput")
    wgu = nc.dram_tensor("wgu", [28, P, KT * P], F16, kind="ExternalInput")
    wd = nc.dram_tensor("wd", [KT, P, FKT * P], F16, kind="ExternalInput")
    wout = nc.dram_tensor("wout", [P, 4 * VC * P], F32, kind="ExternalInput")
    bout = nc.dram_tensor("bout", [P, VC], F32, kind="ExternalInput")
    x0t = nc.dram_tensor("x0t", [P, KT * T], F32, kind="ExternalInput")
    ropebc = nc.dram_tensor("ropebc", [GEN, T, 2 * P], F32,
                            kind="ExternalInput")
    emb = nc.dram_tensor("emb", [V, D], F32, kind="ExternalInput")

    toks_out = nc.dram_tensor("toks", [T, GEN], mybir.dt.int32,
                              kind="ExternalOutput")

    rg = [list(range(NCORES))]

    with tile.TileContext(nc) as tc:
        with (
            tc.tile_pool(name="resident", bufs=1) as res,
            tc.tile_pool(name="acts", bufs=2) as acts,
            tc.tile_pool(name="small", bufs=2) as small,
            tc.tile_pool(name="wqkv_p", bufs=4) as wqkv_pool,
            tc.tile_pool(name="wgu_p", bufs=5) as wgu_pool,
            tc.tile_pool(name="wo_p", bufs=8) as wo_pool,
            tc.tile_pool(name="wd_p", bufs=16) as wd_pool,
            tc.tile_pool(name="psA", bufs=3, space="PSUM") as psA,
            tc.tile_pool(name="psT", bufs=2, space="PSUM") as psT,
            tc.tile_pool(name="psS", bufs=1, space="PSUM") as psS,
            tc.tile_pool(name="dram", bufs=2, space="DRAM") as dram,
        ):
            # ======== one-time init ========
            ident = res.tile([P, P], F32)
            make_identity(nc, ident[:])
            ones_b = res.tile([1, P], F32)      # lhsT for partition-broadcast
            nc.vector.memset(ones_b[:], 1.0)
            ones_k = res.tile([P, 1], mybir.dt.bfloat16)  # lhsT partition-sum
            nc.vector.memset(ones_k[:], 1.0)
            eps_sb = res.tile([1, 1], F32)
            nc.vector.memset(eps_sb[:], EPS)
            eps32 = res.tile([T, 1], F32)
            nc.vector.memset(eps32[:], EPS)

            wout_t = res.tile([P, 4, VC, P], F32)
            nc.sync.dma_start(wout_t[:], wout.ap().rearrange(
                "p (k m q) -> p k m q", k=4, m=VC))
            bout_t = res.tile([P, VC], F32)
            nc.sync.dma_start(bout_t[:], bout.ap())

            kcache = res.tile([T, GEN, P], F32)   # rotated k for kv-head c
            vcache = res.tile([T, GEN, P], F32)
            toks_sb = res.tile([T, GEN], mybir.dt.int32)

            xT = acts.tile([P, KT, T], F32, tag="xT", bufs=1)
            nc.sync.dma_start(xT[:], x0t.ap().rearrange(
                "p (k t) -> p k t", k=KT))

            def rms_norm_T(src):
                """T-layout rms_norm: returns normalized fp16 [P, KT, T]."""
                sq = acts.tile([P, KT, T], mybir.dt.bfloat16, tag="xp8", bufs=1)
                nc.vector.tensor_mul(sq[:], src[:], src[:])
                ssum = psS.tile([1, T], F32, tag="ssum", bufs=1)
                for k in range(KT):
                    nc.tensor.matmul(ssum[:], lhsT=ones_k[:], rhs=sq[:, k, :],
                                     start=(k == 0), stop=(k == KT - 1))
                sgam = small.tile([1, T], F32, tag="sgam", bufs=1)
                nc.scalar.activation(sgam[:], ssum[:],
                                     mybir.ActivationFunctionType.Sqrt,
                                     bias=eps_sb[:], scale=1.0 / D)
                nc.vector.reciprocal(sgam[:], sgam[:])
                sb2 = psS.tile([P, T], F32, tag="bc")
                nc.tensor.matmul(sb2[:], lhsT=ones_b[:], rhs=sgam[:],
                                 start=True, stop=True)
                sbb = small.tile([P, T], F32, tag="sbb", bufs=1)
                nc.vector.tensor_copy(sbb[:], sb2[:])
                hN = acts.tile([P, KT, T], F16, tag="hN", bufs=1)
                nc.vector.tensor_tensor(
                    hN[:], src[:], sbb[:, None, :].to_broadcast([P, KT, T]),
                    op=mybir.AluOpType.mult)
                return hN

            # ---- software-pipelined weight streaming (SP queue only) ----
            from collections import deque
            wq_q, wo_q, wgu_q, wd_q = deque(), deque(), deque(), deque()

            def issue_wqkv(j):
                wt = wqkv_pool.tile([P, KT, P], F16, tag="wqkv")
                nc.sync.dma_start(wt[:], wqkv.ap()[j].rearrange(
                    "p (k q) -> p k q", k=KT))
                wq_q.append(wt)

            def issue_wo(r):
                wt = wo_pool.tile([P, 4, P], F16, tag="wo_w")
                nc.sync.dma_start(wt[:], wo.ap()[r].rearrange(
                    "p (k q) -> p k q", k=4))
                wo_q.append(wt)

            def issue_wgu(r):
                wt = wgu_pool.tile([P, KT, P], F16, tag="wgu")
                nc.sync.dma_start(wt[:], wgu.ap()[r].rearrange(
                    "p (k q) -> p k q", k=KT))
                wgu_q.append(wt)

            def issue_wd(r):
                wt = wd_pool.tile([P, FKT, P], F16, tag="wd_w")
                nc.sync.dma_start(wt[:], wd.ap()[r].rearrange(
                    "p (k q) -> p k q", k=FKT))
                wd_q.append(wt)

            QKV_PF, WO_PF, WGU_PF, WD_PF = 4, 8, 5, 16

            def prefetch_block():
                for j in range(QKV_PF):
                    issue_wqkv(j)
                for r in range(WO_PF):
                    issue_wo(r)
                for r in range(WGU_PF):
                    issue_wgu(r)
                for r in range(WD_PF):
                    issue_wd(r)

            prefetch_block()  # step 0

            # ======== the 8 autoregressive steps ========
            hN_pre = None
            for step in range(GEN):
                S = step + 1  # valid key positions 0..step

                hN = hN_pre if hN_pre is not None else rms_norm_T(xT)

                # ---- QKV projections: 6 regions in one psum bank ----
                pqkv = psA.tile([P, 6, T], F32, tag="mm")
                for j in range(6):
                    if j + QKV_PF < 6:
                        issue_wqkv(j + QKV_PF)
                    wt = wq_q.popleft()
                    for k in range(KT):
                        nc.tensor.matmul(pqkv[:, j, :], lhsT=wt[:, k, :],
                                         rhs=hN[:, k, :],
                                         start=(j == 0 and k == 0),
                                         stop=(j == 5 and k == KT - 1))
                qkvT = small.tile([P, 6, T], F32, tag="ev", bufs=1)
                nc.vector.tensor_copy(qkvT[:], pqkv[:])

                # ---- to normal layout [tok, feat]: q heads + k batched ----
                qr = small.tile([T, 5, P], F32, tag="erow", bufs=1)
                for j in range(5):
                    tp = psT.tile([T, P], F32, tag="tp")
                    nc.tensor.transpose(tp[:], qkvT[:, j, :], ident[:])
                    nc.vector.tensor_copy(qr[:, j, :], tp[:])
                tpv = psT.tile([T, P], F32, tag="tp")
                nc.tensor.transpose(tpv[:], qkvT[:, 5, :], ident[:])
                nc.vector.tensor_copy(vcache[:, step, :], tpv[:])

                # ---- RoPE on q (4 heads) + k, batched + in place ----
                # per-step broadcast tables streamed from DRAM (tiny load)
                cs_t = small.tile([T, 2, P], F32, tag="cosi", bufs=1)
                nc.scalar.dma_start(cs_t[:], ropebc.ap()[step])
                H2 = HD // 2
                co = cs_t[:, 0, :]
                si = cs_t[:, 1, :]
                t2 = small.tile([T, 5, P], F32, tag="pr", bufs=1)
                nc.vector.tensor_tensor(
                    t2[:, :, :H2], qr[:, :, H2:],
                    si[:, None, :H2].to_broadcast([T, 5, H2]),
                    op=mybir.AluOpType.mult)
                nc.vector.tensor_tensor(
                    t2[:, :, H2:], qr[:, :, :H2],
                    si[:, None, H2:].to_broadcast([T, 5, H2]),
                    op=mybir.AluOpType.mult)
                nc.vector.tensor_tensor(
                    qr[:], qr[:],
                    co[:, None, :].to_broadcast([T, 5, P]),
                    op=mybir.AluOpType.mult)
                nc.vector.tensor_tensor(qr[:, :, :H2], qr[:, :, :H2],
                                        t2[:, :, :H2],
                                        op=mybir.AluOpType.subtract)
                nc.vector.tensor_add(qr[:, :, H2:], qr[:, :, H2:],
                                     t2[:, :, H2:])
                nc.vector.tensor_copy(kcache[:, step, :], qr[:, 4, :])

                # ---- attention (DVE, normal layout, S keys) ----
                sc = small.tile([T, QH, GEN], F32, tag="sc", bufs=1)
                for j0 in range(0, S, 2):
                    cnt = min(2, S - j0)
                    bat = small.tile([T, 2, QH, P], F32, tag="pr", bufs=1)
                    nc.vector.tensor_tensor(
                        bat[:, :cnt],
                        qr[:, None, :QH, :].to_broadcast([T, cnt, QH, P]),
                        kcache[:, j0:j0 + cnt, None, :].to_broadcast(
                            [T, cnt, QH, P]),
                        op=mybir.AluOpType.mult)
                    nc.vector.tensor_reduce(
                        sc[:, :, j0:j0 + cnt].rearrange("t q s -> t s q"),
                        bat[:, :cnt], axis=mybir.AxisListType.X,
                        op=mybir.AluOpType.add)
                # no max-subtraction: scores are bounded (|sc| ~< 10) so
                # exp is safe in fp32 and the normalization cancels it
                es = small.tile([T, QH, GEN], F32, tag="es", bufs=1)
                nc.scalar.activation(es[:, :, :S], sc[:, :, :S],
                                     mybir.ActivationFunctionType.Exp)
                sm = small.tile([T, QH], F32, tag="sm", bufs=1)
                nc.vector.reduce_sum(sm[:], es[:, :, :S],
                                     axis=mybir.AxisListType.X)
                nc.vector.reciprocal(sm[:], sm[:])
                nc.vector.tensor_tensor(
                    es[:, :, :S], es[:, :, :S],
                    sm[:, :, None].to_broadcast([T, QH, S]),
                    op=mybir.AluOpType.mult)
                ao = small.tile([T, QH, P], F32, tag="lgN", bufs=1)
                aofirst = None
                for j in range(S):
                    contrib = small.tile([T, QH, P], F32, tag="contrib",
                                         bufs=2)
                    nc.vector.tensor_tensor(
                        contrib[:],
                        es[:, :, j, None].to_broadcast([T, QH, P]),
                        vcache[:, j, None, :].to_broadcast([T, QH, P]),
                        op=mybir.AluOpType.mult)
                    if j == 0:
                        aofirst = contrib
                    elif j == 1:
                        nc.vector.tensor_add(ao[:], aofirst[:], contrib[:])
                    else:
                        nc.vector.tensor_add(ao[:], ao[:], contrib[:])
                if S == 1:
                    nc.vector.tensor_copy(ao[:], aofirst[:])

                # ---- transpose ao back to T layout (fp16 for the matmul) ----
                aoT = small.tile([P, QH, T], F16, tag="aoT", bufs=1)
                for j in range(QH):
                    tp2 = psT.tile([P, T], F32, tag="tp")
                    nc.tensor.transpose(tp2[:], ao[:, j, :], ident[:T, :T])
                    nc.vector.tensor_copy(aoT[:, j, :], tp2[:])

                # ---- Wo partial (row-parallel): 32 regions, 2 banks ----
                arin = dram.tile([P, KT * T], F32, tag="arin")
                for g in range(2):
                    pw = psA.tile([P, 16 * T], F32, tag="mm")
                    for mt in range(16):
                        r = g * 16 + mt
                        if r + WO_PF < KT:
                            issue_wo(r + WO_PF)
                        wt = wo_q.popleft()
                        for k4 in range(4):
                            nc.tensor.matmul(pw[:, mt * T:(mt + 1) * T],
                                             lhsT=wt[:, k4, :],
                                             rhs=aoT[:, k4, :],
                                             start=(mt == 0 and k4 == 0),
                                             stop=(mt == 15 and k4 == 3))
                    ev = small.tile([P, 16 * T], F32, tag="ev", bufs=1)
                    nc.vector.tensor_copy(ev[:], pw[:])
                    nc.scalar.dma_start(
                        arin[:, g * 16 * T:(g + 1) * 16 * T], ev[:])
                # RS + AG (numerically = AllReduce, cheaper in cost model);
                # rs1 must NOT be Shared (collectives can't read Shared).
                rs1 = dram.tile([P // NCORES, KT * T], F32, tag="rs1")
                nc.gpsimd.collective_compute(
                    "ReduceScatter", mybir.AluOpType.add, replica_groups=rg,
                    ins=[arin[:]], outs=[rs1[:]])
                arout = dram.tile([P, KT * T], F32, tag="arout",
                                  addr_space="Shared")
                nc.gpsimd.collective_compute(
                    "AllGather", mybir.AluOpType.bypass, replica_groups=rg,
                    ins=[rs1[:]], outs=[arout[:]])
                # accumulate the AllGathered attention output into xT in place
                # (accum DMA requires the gpsimd software-DGE path)
                nc.gpsimd.dma_start(
                    xT[:], arout.rearrange("p (k t) -> p k t", k=KT),
                    accum_op=mybir.AluOpType.add)

                h2N = rms_norm_T(xT)
                # xp/8 folded into the Wd partial before the ReduceScatter
                xp8 = acts.tile([P, KT, T], F32, tag="xp8", bufs=1)
                nc.vector.tensor_scalar(xp8[:], xT[:], scalar1=1.0 / NCORES,
                                        scalar2=0.0,
                                        op0=mybir.AluOpType.mult,
                                        op1=mybir.AluOpType.add)

                # ---- MLP up: 28 regions -> 2 banks of 14 ----
                pgu_a = psA.tile([P, 14 * T], F32, tag="mm")
                pgu_b = psA.tile([P, 14 * T], F32, tag="mm")
                for r in range(28):
                    pg = pgu_a if r < 14 else pgu_b
                    jj = r % 14
                    if r + WGU_PF < 28:
                        issue_wgu(r + WGU_PF)
                    wt = wgu_q.popleft()
                    for k in range(KT):
                        nc.tensor.matmul(pg[:, jj * T:(jj + 1) * T],
                                         lhsT=wt[:, k, :],
                                         rhs=h2N[:, k, :],
                                         start=(jj == 0 and k == 0),
                                         stop=(jj == 13 and k == KT - 1))
                gS = small.tile([P, 14 * T], F32, tag="erow", bufs=1)
                nc.scalar.activation(gS[:], pgu_a[:],
                                     mybir.ActivationFunctionType.Silu)
                mT = small.tile([P, FKT, T], F16, tag="lgN", bufs=1)
                nc.vector.tensor_tensor(
                    mT[:].rearrange("p k t -> p (k t)"), gS[:], pgu_b[:],
                    op=mybir.AluOpType.mult)

                # ---- MLP down partial + xp/8: 32 regions, 2 banks ----
                arin2 = dram.tile([P, KT * T], F32, tag="arin")
                for g in range(2):
                    pd = psA.tile([P, 16 * T], F32, tag="mm")
                    for mt in range(16):
                        r = g * 16 + mt
                        if r + WD_PF < KT:
                            issue_wd(r + WD_PF)
                        wt = wd_q.popleft()
                        for k in range(FKT):
                            nc.tensor.matmul(pd[:, mt * T:(mt + 1) * T],
                                             lhsT=wt[:, k, :],
                                             rhs=mT[:, k, :],
                                             start=(mt == 0 and k == 0),
                                             stop=(mt == 15 and k == FKT - 1))
                    ev2 = small.tile([P, 16 * T], F32, tag="ev", bufs=1)
                    nc.vector.tensor_add(
                        ev2[:], pd[:],
                        xp8[:, g * 16:(g + 1) * 16, :].rearrange(
                            "p r t -> p (r t)"))
                    nc.scalar.dma_start(
                        arin2[:, g * 16 * T:(g + 1) * 16 * T], ev2[:])
                # prefetch next step's leading weight tiles: streams during
                # the logits/argmax/emb tail and the next attention+RS+AG
                if step < GEN - 1:
                    prefetch_block()
                # ReduceScatter: core c receives its feature slice of
                # xo = xp + mlp, features Fc = {128r + p : p in [16c,16c+16)}
                rs2 = dram.tile([P // NCORES, KT * T], F32, tag="rs2")
                nc.gpsimd.collective_compute(
                    "ReduceScatter", mybir.AluOpType.add, replica_groups=rg,
                    ins=[arin2[:]], outs=[rs2[:]])

                # ---- logits (fp32, col-sharded resident W_out) ----
                # xo chunk j: rows p' in [4j,4j+4) x all 32 ktiles;
                # partition q = (p'-4j)*32 + r, feature f = 128r + 16c + p'
                xoc = small.tile([P, 4, T], F32, tag="xoc", bufs=1)
                rs2v = rs2.rearrange("p (r t) -> p r t", r=KT)
                for j in range(4):
                    nc.scalar.dma_start(
                        xoc[:, j, :],
                        rs2v[4 * j:4 * (j + 1), :, :].rearrange(
                            "p r t -> (p r) t"))
                pl = psA.tile([P, VC, T], F32, tag="mm")
                for j in range(4):
                    for m in range(VC):
                        nc.tensor.matmul(pl[:, m, :],
                                         lhsT=wout_t[:, j, m, :],
                                         rhs=xoc[:, j, :],
                                         start=(j == 0 and m == 0),
                                         stop=(j == 3 and m == VC - 1))
                plog = small.tile([P, VC, T], F32, tag="ev", bufs=1)
                nc.vector.tensor_tensor(
                    plog[:], pl[:],
                    bout_t[:, :, None].to_broadcast([P, VC, T]),
                    op=mybir.AluOpType.add)
                # transpose to token-major BEFORE the collective so the
                # ReduceScatter shards by token (4 tokens per core)
                lgN = small.tile([T, V], F32, tag="lgN", bufs=1)
                for h in range(2):
                    plN = psT.tile([T, 4, P], F32, tag="tbig", bufs=1)
                    for m4 in range(4):
                        nc.tensor.transpose(plN[:, m4, :],
                                            plog[:, h * 4 + m4, :], ident[:])
                    nc.vector.tensor_copy(
                        lgN[:, h * 4 * P:(h + 1) * 4 * P].rearrange(
                            "t (m q) -> t m q", m=4), plN[:])
                lrs_in = dram.tile([T, V], F32, tag="lrs_in")
                nc.scalar.dma_start(lrs_in[:], lgN[:])
                lrs_out = dram.tile([T // NCORES, V], F32, tag="lrs_out")
                nc.gpsimd.collective_compute(
                    "ReduceScatter", mybir.AluOpType.add, replica_groups=rg,
                    ins=[lrs_in[:]], outs=[lrs_out[:]])
                lg4 = small.tile([T // NCORES, V], F32, tag="lgN", bufs=1)
                nc.scalar.dma_start(lg4[:], lrs_out[:])
                # local argmax of this core's 4 tokens, then AllGather ids
                v8 = small.tile([T // NCORES, 8], F32, tag="v8", bufs=1)
                i8 = small.tile([T // NCORES, 8], mybir.dt.uint32,
                                tag="i8", bufs=1)
                nc.vector.max_with_indices(v8[:], i8[:], lg4[:])
                idf = small.tile([T // NCORES, 1], F32, tag="idf", bufs=1)
                nc.vector.tensor_copy(idf[:], i8[:, 0:1])
                ids_in = dram.tile([T // NCORES, 1], F32, tag="ids_in")
                nc.scalar.dma_start(ids_in[:], idf[:])
                ids_out = dram.tile([T, 1], F32, tag="ids_out",
                                    addr_space="Shared")
                nc.gpsimd.collective_compute(
                    "AllGather", mybir.AluOpType.bypass, replica_groups=rg,
                    ins=[ids_in[:]], outs=[ids_out[:]])
                tokf = small.tile([T, 1], F32, tag="tokf", bufs=1)
                nc.scalar.dma_start(tokf[:], ids_out[:])
                toku = small.tile([T, 1], mybir.dt.uint32, tag="toku", bufs=1)
                nc.vector.tensor_copy(toku[:], tokf[:])
                nc.vector.tensor_copy(toks_sb[:, step, None], toku[:])

                # ---- embedding gather + transpose into next-step xT,
                # with ln1 rms-norm fused into the evacuation ----
                if step < GEN - 1:
                    erow = small.tile([T, D], F32, tag="erow", bufs=1)
                    nc.gpsimd.indirect_dma_start(
                        out=erow[:], out_offset=None, in_=emb.ap(),
                        in_offset=bass.IndirectOffsetOnAxis(
                            ap=toku[:, :1], axis=0))
                    # sum of squares per token (token-major: free-dim reduce)
                    sums4 = small.tile([T, 4], F32, tag="sums4", bufs=1)
                    junk = small.tile([T, 1024], mybir.dt.bfloat16,
                                      tag="pr", bufs=1)
                    for cch in range(4):
                        nc.scalar.activation(
                            junk[:], erow[:, cch * 1024:(cch + 1) * 1024],
                            mybir.ActivationFunctionType.Square,
                            accum_out=sums4[:, cch:cch + 1])
                    ssn = small.tile([T, 1], F32, tag="ssn", bufs=1)
                    nc.vector.reduce_sum(ssn[:], sums4[:],
                                         axis=mybir.AxisListType.X)
                    nc.scalar.activation(ssn[:], ssn[:],
                                         mybir.ActivationFunctionType.Sqrt,
                                         bias=eps32[:], scale=1.0 / D)
                    nc.vector.reciprocal(ssn[:], ssn[:])
                    tps = psS.tile([P, T], F32, tag="bc")
                    nc.tensor.transpose(tps[:1, :T], ssn[:], ident[:T, :T])
                    srow = small.tile([1, T], F32, tag="srow", bufs=1)
                    nc.vector.tensor_copy(srow[:], tps[:1, :T])
                    sb4 = psS.tile([P, T], F32, tag="bc")
                    nc.tensor.matmul(sb4[:], lhsT=ones_b[:], rhs=srow[:],
                                     start=True, stop=True)
                    sbbN = small.tile([P, T], F32, tag="sbb", bufs=1)
                    nc.vector.tensor_copy(sbbN[:], sb4[:])
                    xTn = acts.tile([P, KT, T], F32, tag="xT", bufs=1)
                    hNn = acts.tile([P, KT, T], F16, tag="hN", bufs=1)
                    for h in range(2):
                        tpe = psT.tile([P, 16, T], F32, tag="tbig", bufs=1)
                        for k16 in range(16):
                            k = h * 16 + k16
                            nc.tensor.transpose(
                                tpe[:, k16, :], erow[:, k * P:(k + 1) * P],
                                ident[:T, :T])
                        nc.vector.tensor_copy(xTn[:, h * 16:(h + 1) * 16, :],
                                              tpe[:])
                        nc.vector.tensor_tensor(
                            hNn[:, h * 16:(h + 1) * 16, :], tpe[:],
                            sbbN[:, None, :].to_broadcast([P, 16, T]),
                            op=mybir.AluOpType.mult)
                    xT = xTn
                    hN_pre = hNn

            nc.sync.dma_start(toks_out.ap(), toks_sb[:])

    nc.compile()
    nc.finalize()
    return nc


def _pack_inputs(inputs):
    """Build the 8 per-core input maps from the full (unsharded) inputs."""
    Wq = np.asarray(inputs["Wq"], np.float32)
    Wk = np.asarray(inputs["Wk"], np.float32)
    Wv = np.asarray(inputs["Wv"], np.float32)
    Wo = np.asarray(inputs["Wo"], np.float32)
    Wg = np.asarray(inputs["Wg"], np.float32)
    Wu = np.asarray(inputs["Wu"], np.float32)
    Wd = np.asarray(inputs["Wd"], np.float32)
    W_out = np.asarray(inputs["W_out"], np.float32)
    b_out = np.asarray(inputs["b_out"], np.float32)
    w_ln1 = np.asarray(inputs["w_ln1"], np.float32)
    w_ln2 = np.asarray(inputs["w_ln2"], np.float32)
    emb = np.ascontiguousarray(np.asarray(inputs["emb"], np.float32))
    x0 = np.asarray(inputs["chunk_hidden_states"], np.float32)[0]  # [T, D]

    Wq_s = Wq * w_ln1[None, :] * np.float32(1.0 / np.sqrt(np.float32(HD)))
    Wk_s = Wk * w_ln1[None, :]
    Wv_s = Wv * w_ln1[None, :]
    Wg_s = Wg * w_ln2[None, :]
    Wu_s = Wu * w_ln2[None, :]

    # rope tables at positions 0..GEN-1 (fp32, matching reference),
    # pre-broadcast over the T tokens: [GEN, T, 2*P]
    inv = 1.0 / (ROPE_THETA ** (np.arange(0, HD, 2, dtype=np.float32) / HD))
    freqs = np.arange(GEN, dtype=np.float32)[:, None] * inv[None, :]
    embf = np.concatenate([freqs, freqs], axis=-1)
    cs = np.concatenate(
        [np.cos(embf), np.sin(embf)], axis=-1).astype(np.float32)  # [GEN, 2P]
    ropebc = np.ascontiguousarray(
        np.broadcast_to(cs[:, None, :], (GEN, T, 2 * P)))

    x0t = np.ascontiguousarray(
        x0.T.reshape(KT, P, T).transpose(1, 0, 2).reshape(P, KT * T))

    def regpack(Wmat):
        """[R*128 outfeat, KIN] -> [R, 128 p(kin-tile-row), KIN/128*128]
        where block r, element [p, k*128+q] = Wmat[r*128+q, k*128+p]."""
        R = Wmat.shape[0] // P
        KIN = Wmat.shape[1]
        KTl = KIN // P
        arr = Wmat.reshape(R, P, KTl, P).transpose(0, 3, 2, 1)  # r,p,k,q
        return np.ascontiguousarray(arr).reshape(R, P, KTl * P)

    in_maps = []
    for c in range(NCORES):
        wq_r = regpack(Wq_s[512 * c:512 * (c + 1)])      # [4, 128, 4096]
        wk_r = regpack(Wk_s[P * c:P * (c + 1)])          # [1, 128, 4096]
        wv_r = regpack(Wv_s[P * c:P * (c + 1)])
        wqkv = np.concatenate([wq_r, wk_r, wv_r], axis=0).astype(np.float16)

        # Wo: out rows = D (32 regions), contraction = this core's 512 cols
        wo_pack = regpack(
            np.ascontiguousarray(Wo[:, 512 * c:512 * (c + 1)])
        ).astype(np.float16)                              # [32,128,512]

        wg_r = regpack(Wg_s[FFC * c:FFC * (c + 1)])      # [14, 128, 4096]
        wu_r = regpack(Wu_s[FFC * c:FFC * (c + 1)])
        wgu = np.concatenate([wg_r, wu_r], axis=0).astype(np.float16)

        wd_pack = regpack(
            np.ascontiguousarray(Wd[:, FFC * c:FFC * (c + 1)])
        ).astype(np.float16)                              # [32,128,1792]

        # W_out column shard for features Fc = {128r + p : p in [16c,16c+16)}
        # lhsT chunk (j, m): [q, v] = W_out[128m+v, f(j,q)],
        # f(j, q) = 128*(q % 32) + 16c + 4j + q//32
        q_idx = np.arange(P)
        wout_pack = np.empty((P, 4, VC, P), np.float32)
        for j in range(4):
            f = 128 * (q_idx % 32) + 16 * c + 4 * j + q_idx // 32  # [128]
            # wout_pack[q, j, m, v] = W_out[128m+v, f[q]]
            Wslice = W_out[:, f]                  # [V, 128] -> (128m+v, q)
            wout_pack[:, j] = Wslice.reshape(VC, P, P).transpose(2, 0, 1)
        wout_pack = np.ascontiguousarray(wout_pack).reshape(P, 4 * VC * P)
        # bias/8 packed [v, m]
        bout_pack = np.ascontiguousarray(
            (b_out / NCORES).reshape(VC, P).T)            # [128, 8]

        in_maps.append({
            "wqkv": np.ascontiguousarray(wqkv),
            "wo": np.ascontiguousarray(wo_pack),
            "wgu": np.ascontiguousarray(wgu),
            "wd": np.ascontiguousarray(wd_pack),
            "wout": wout_pack,
            "bout": bout_pack,
            "x0t": x0t,
            "ropebc": ropebc,
            "emb": emb,
        })
    return in_maps


def kernel(**inputs) -> np.ndarray:
    if "nc" not in _CACHED:
        _CACHED["nc"] = _build_nc()
    nc = _CACHED["nc"]
    in_maps = _pack_inputs(inputs)
    res = run_bass_kernel_spmd(nc, in_maps, core_ids=list(range(NCORES)))
    return np.asarray(res.results[0]["toks"], np.int32)
